# revision 1
# baseline (speedup 1.0000x reference)
"""AConnect (nn_AConnect_82368882803074) Trainium2 kernel.

Reference computation:
    memW[b]    = W * Werr_bank[idx[b]]             [B, D_in, D_out]
    membias[b] = bias * Berr_bank[idx[b]]          [B, 1, D_out]
    Z[b]       = X[b] @ memW[b] + membias[b]       [B, D_out]

Strategy: data-parallel over the batch. Each of the 8 NeuronCores gets 32
samples. The host shards X/idx and gathers each core's 32 indexed error-bank
matrices (pure data movement); all arithmetic (W ⊙ E multiply, the batched
vector-matrix reduction, and the bias term) runs on device.

Per core the device kernel streams the 32 gathered 1 MB bank matrices from
HBM (the memory-roofline term, ~32 MB/core), multiplies by W on VectorE, and
contracts with X[b] on TensorE (k-chunked matmuls accumulating in PSUM).
"""

import os

import numpy as np

B, D_IN, D_OUT, N_BANK, N_CORES = 256, 512, 512, 1000, 8
S = B // N_CORES  # 32 samples per core
P = 128  # partitions
C = D_IN // P  # 4 k-chunks

_CACHE = {}
last_exec_time_ns = None


def _build_nc():
    import concourse.mybir as mybir
    import concourse.tile as tile
    from concourse import bacc

    f32 = mybir.dt.float32
    nc = bacc.Bacc()

    eg = nc.dram_tensor("eg", [S, P, C * D_OUT], f32, kind="ExternalInput")
    wt = nc.dram_tensor("wt", [P, C * D_OUT], f32, kind="ExternalInput")
    xtt = nc.dram_tensor("xtt", [P, C * S], f32, kind="ExternalInput")
    bb = nc.dram_tensor("bb", [S, D_OUT], f32, kind="ExternalInput")
    beg = nc.dram_tensor("beg", [S, D_OUT], f32, kind="ExternalInput")
    out = nc.dram_tensor("out", [S, D_OUT], f32, kind="ExternalOutput")

    with tile.TileContext(nc) as tc:
        with (
            tc.tile_pool(name="const", bufs=1) as constp,
            tc.tile_pool(name="ep", bufs=4) as ep,
            tc.tile_pool(name="wep", bufs=4) as wep,
            tc.tile_pool(name="ps", bufs=4, space="PSUM") as psp,
            tc.tile_pool(name="outp", bufs=1) as outp,
        ):
            w_t = constp.tile([P, C * D_OUT], f32)
            nc.sync.dma_start(w_t[:], wt[:])
            x_t = constp.tile([P, C * S], f32)
            nc.sync.dma_start(x_t[:], xtt[:])
            bias_t = constp.tile([S, D_OUT], f32)
            nc.sync.dma_start(bias_t[:], bb[:])
            berr_t = constp.tile([S, D_OUT], f32)
            nc.sync.dma_start(berr_t[:], beg[:])
            mb = constp.tile([S, D_OUT], f32)
            nc.vector.tensor_mul(mb[:], bias_t[:], berr_t[:])

            zstage = outp.tile([1, S * D_OUT], f32)
            for s in range(S):
                e = ep.tile([P, C * D_OUT], f32)
                nc.sync.dma_start(e[:], eg[s])
                we = wep.tile([P, C * D_OUT], f32)
                nc.vector.tensor_mul(we[:], e[:], w_t[:])
                ps = psp.tile([1, D_OUT], f32)
                for c in range(C):
                    nc.tensor.matmul(
                        ps[:],
                        x_t[:, c * S + s : c * S + s + 1],
                        we[:, c * D_OUT : (c + 1) * D_OUT],
                        start=(c == 0),
                        stop=(c == C - 1),
                    )
                nc.any.tensor_copy(zstage[0:1, s * D_OUT : (s + 1) * D_OUT], ps[:])

            zre = outp.tile([S, D_OUT], f32)
            nc.sync.dma_start(
                zre[:], zstage[0:1, :].rearrange("p (s n) -> p s n", n=D_OUT)
            )
            fin = outp.tile([S, D_OUT], f32)
            nc.vector.tensor_add(fin[:], zre[:], mb[:])
            nc.sync.dma_start(out[:], fin[:])

    nc.compile()
    return nc


def _install_trace_shim():
    """Register the axon NTFF profile hook bass_utils expects (the agent
    image lacks antenv.axon_hooks; the C ABI is in libaxon_pjrt.so)."""
    import contextlib
    import ctypes
    import sys
    import types

    if "antenv.axon_hooks" in sys.modules:
        return
    lib = ctypes.CDLL("/opt/axon/libaxon_pjrt.so")
    if not hasattr(lib, "axon_start_nrt_profile"):
        hook = None
    else:
        lib.axon_start_nrt_profile.argtypes = [
            ctypes.POINTER(ctypes.c_int64),
            ctypes.c_size_t,
        ]
        lib.axon_start_nrt_profile.restype = ctypes.c_int64
        lib.axon_stop_nrt_profile.argtypes = [ctypes.c_char_p]
        lib.axon_stop_nrt_profile.restype = ctypes.c_int64

        @contextlib.contextmanager
        def hook(output_dir, device_ids):
            import jax

            jax.devices()
            if device_ids:
                ids = (ctypes.c_int64 * len(device_ids))(*device_ids)
                rc = lib.axon_start_nrt_profile(ids, len(device_ids))
            else:
                rc = lib.axon_start_nrt_profile(None, 0)
            if rc != 0:
                raise RuntimeError(f"axon_start_nrt_profile rc={rc}")
            try:
                yield
            finally:
                n = lib.axon_stop_nrt_profile(str(output_dir).encode())
                print(f"ntff profile: {n} file(s) -> {output_dir}", file=sys.stderr)

    mod = types.ModuleType("antenv.axon_hooks")
    mod.get_axon_ntff_profile_hook = lambda: hook
    mod.set_axon_ntff_profile_hook = lambda h: None
    sys.modules["antenv.axon_hooks"] = mod


def kernel(X, W, bias, Werr_bank, Berr_bank, idx):
    global last_exec_time_ns
    from concourse.bass_utils import run_bass_kernel_spmd

    X = np.asarray(X, dtype=np.float32)
    W = np.asarray(W, dtype=np.float32)
    bias = np.asarray(bias, dtype=np.float32)
    Werr_bank = np.asarray(Werr_bank, dtype=np.float32)
    Berr_bank = np.asarray(Berr_bank, dtype=np.float32)
    idx = np.asarray(idx, dtype=np.int32)

    if "nc" not in _CACHE:
        _CACHE["nc"] = _build_nc()
    nc = _CACHE["nc"]

    # Host-side sharding / layout (pure data movement).
    wt = np.ascontiguousarray(
        W.reshape(C, P, D_OUT).transpose(1, 0, 2).reshape(P, C * D_OUT)
    )
    bb = np.ascontiguousarray(np.broadcast_to(bias.reshape(1, D_OUT), (S, D_OUT)))

    in_maps = []
    for c_id in range(N_CORES):
        sl = slice(c_id * S, (c_id + 1) * S)
        idx_s = idx[sl]
        gath = Werr_bank[idx_s]  # [S, D_in, D_out]
        eg = np.ascontiguousarray(
            gath.reshape(S, C, P, D_OUT).transpose(0, 2, 1, 3).reshape(S, P, C * D_OUT)
        )
        xs = X[sl]  # [S, D_in]
        xtt = np.ascontiguousarray(
            xs.T.reshape(C, P, S).transpose(1, 0, 2).reshape(P, C * S)
        )
        beg = np.ascontiguousarray(Berr_bank[idx_s, 0, :])
        in_maps.append({"eg": eg, "wt": wt, "xtt": xtt, "bb": bb, "beg": beg})

    trace = os.environ.get("BASS_KERNEL_TRACE") == "1"
    if trace:
        _install_trace_shim()
    res = run_bass_kernel_spmd(
        nc,
        in_maps,
        core_ids=list(range(N_CORES)),
        trace=trace,
        trace_cores=[0] if trace else None,
    )
    last_exec_time_ns = res.exec_time_ns
    return np.concatenate([r["out"] for r in res.results], axis=0)
